# revision 15
# baseline (speedup 1.0000x reference)
"""GCN message-passing kernel for Trainium2, 8 NeuronCores (v6).

Math (reference): 3-layer GCN with symmetric normalization and self-loops,
then dot-product decode over label edge pairs.

Reformulation: A_hat @ (x @ W) == (A_hat @ x) @ W, so each layer is
  agg = A_hat @ z          (sparse gather + PE indicator-matmul scatter)
  z   = relu(agg @ W + b)
A_hat is shared by all 3 layers; normalization folded into per-edge values.

v6 design:
- fp16 data path; z3 + decode fp32. Indicators precomputed host-side and
  streamed from DRAM (HWDGE), zero per-chunk DVE work.
- The critical resource is GpSimd Q7 descriptor emission (~3-5ns/desc,
  serial). Everything is organized to keep it busy continuously:
  * z is published in TWO segments (A = each core's first 30 blocks,
    B = rest) via two AllGathers into SEPARATE shared tensors zfA/zfB.
    AG_A fires mid-layer; the NEXT layer's A-chunk gathers depend only
    on zfA, so their descriptor emission overlaps the current layer's
    tail compute and AG_B.
  * chunks are classed by source segment (A: rows < 30720, B: rest) -
    both segments are int16-addressable relative to their table base,
    replacing the old 32768 low/high split.
- Per-block source dedup: an edge source appearing k times for one
  (block, class) occupies ONE gather slot; its indicator column has k
  nonzeros.
- dst blocks assigned to cores snake-wise by edge count so the SPMD
  program's per-block chunk counts (max over cores) have ~zero padding.
- Self-loops via contiguous HWDGE dma_start from the core's own local
  zs slice + resident diag(dinv^2) indicator (start=True clears psum).
- Decode bucketed by (a-seg, b-seg); the (A,A) bucket is gathered right
  after AG3_A, overlapping layer-3 tail.
"""

import numpy as np

P = 128
N_CORES = 8
BPC = 49                # blocks per core (392 / 8)
GROUPS = [5] * 9 + [4]  # blocks per gather group (sum = 49)
SEG0_BLOCKS = 30        # segment A blocks per core
SA = SEG0_BLOCKS * P            # 3840 local rows in segment A
SB = BPC * P - SA               # 2432 local rows in segment B
NA = N_CORES * SA               # 30720 zf rows in segment A
NB = N_CORES * SB               # 19456 zf rows in segment B
N_NODES_PAD = NA + NB


def _zf_base(core, lrow):
    """zf row of (core, local row) under the segment-major layout."""
    if lrow < SA:
        return core * SA + lrow
    return NA + core * SB + (lrow - SA)


def _wrap16(flat_idx):
    """dma_gather idx layout: idx i at [i%16, i//16], replicated to 128 rows."""
    t = flat_idx.astype(np.int16).reshape(-1, 16).T
    return np.tile(t, (8, 1))


# ---------------------------------------------------------------- host prep

def prepare(edge_index, n_nodes):
    src = np.asarray(edge_index[0], dtype=np.int64)
    dst = np.asarray(edge_index[1], dtype=np.int64)
    deg = np.bincount(dst, minlength=N_NODES_PAD).astype(np.float64) + 1.0
    dinv = (1.0 / np.sqrt(deg)).astype(np.float32)

    n_blocks = N_CORES * BPC
    blk_edges = np.bincount(dst >> 7, minlength=n_blocks)

    order = np.argsort(-blk_edges, kind="stable")
    blk_core = np.empty(n_blocks, np.int64)
    blk_local = np.empty(n_blocks, np.int64)
    for r, b in enumerate(order):
        rnd, pos = divmod(r, N_CORES)
        core = pos if rnd % 2 == 0 else N_CORES - 1 - pos
        blk_core[b] = core
        blk_local[b] = rnd
    lrow = blk_local * P
    seg_start = np.where(lrow < SA,
                         blk_core * SA + lrow,
                         NA + blk_core * SB + (lrow - SA))
    zfrow = (seg_start[np.arange(N_NODES_PAD) >> 7]
             + (np.arange(N_NODES_PAD) & 127))

    srow = zfrow[src]
    drow = zfrow[dst]
    enrm = (dinv[src] * dinv[dst]).astype(np.float32)

    dblk = drow >> 7
    dnl = (drow & 127).astype(np.int32)
    cls = (srow >= NA).astype(np.int64)   # 0 = segment A source, 1 = B

    # dedup: one slot per (block, class, source row)
    so = np.lexsort((srow, cls, dblk))
    srow_s, dnl_s, enrm_s = srow[so], dnl[so], enrm[so]
    dblk_s, cls_s = dblk[so], cls[so]
    newslot = np.ones(len(so), bool)
    newslot[1:] = ((srow_s[1:] != srow_s[:-1]) | (cls_s[1:] != cls_s[:-1])
                   | (dblk_s[1:] != dblk_s[:-1]))
    slot_id = np.cumsum(newslot) - 1          # per-edge unique-slot index
    n_slots = int(slot_id[-1]) + 1
    slot_srow = srow_s[newslot]
    slot_blk = dblk_s[newslot]
    slot_cls = cls_s[newslot]

    # per (block, class) slot counts
    key = slot_blk * 2 + slot_cls
    cnts = np.bincount(key, minlength=2 * n_blocks)
    B_of = np.empty((N_CORES, BPC), np.int64)
    for c in range(N_CORES):
        for i in range(BPC):
            B_of[c, i] = _zf_base(c, i * P) >> 7
    nA = cnts[0::2][B_of]
    nB = cnts[1::2][B_of]
    cntA = np.ceil(nA / P).astype(np.int64).max(axis=0)
    cntB = np.ceil(nB / P).astype(np.int64).max(axis=0)

    C = int((cntA + cntB).sum())
    gidx = np.zeros((N_CORES, C * P), np.int64)

    n_groups = len(GROUPS)
    gstart = np.cumsum([0] + GROUPS)
    posA = np.zeros(BPC, np.int64)
    posB = np.zeros(BPC, np.int64)
    pos = 0
    grp_cntA = np.zeros(n_groups, np.int64)
    grp_cntB = np.zeros(n_groups, np.int64)
    for g in range(n_groups):
        b0, b1 = int(gstart[g]), int(gstart[g + 1])
        for i in range(b0, b1):
            posA[i] = pos
            pos += cntA[i]
        for i in range(b0, b1):
            posB[i] = pos
            pos += cntB[i]
        grp_cntA[g] = cntA[b0:b1].sum()
        grp_cntB[g] = cntB[b0:b1].sum()
    assert pos == C

    # within-(block,class) ordinal of each unique slot
    first_of_key = np.zeros(2 * n_blocks, np.int64)
    kstart = np.searchsorted(key, np.arange(2 * n_blocks))
    within = np.arange(n_slots) - kstart[key]

    ind_edge = np.zeros((N_CORES, P, C * P), np.float16)
    for c in range(N_CORES):
        spos = np.full(2 * n_blocks, -1, np.int64)
        for i in range(BPC):
            b = B_of[c, i]
            spos[2 * b] = posA[i] * P
            spos[2 * b + 1] = posB[i] * P
        slot_pos = np.where(spos[key] >= 0, spos[key] + within, -1)
        m = slot_pos >= 0
        gidx[c, slot_pos[m]] = slot_srow[m] - slot_cls[m] * NA
        es = slot_pos[slot_id]            # per sorted-edge stream position
        ev = es >= 0
        epos = es[ev]
        np.add.at(
            ind_edge[c],
            (epos % P, (epos // P) * P + dnl_s[ev]),
            enrm_s[ev].astype(np.float16))

    eidx = np.stack([_wrap16(gidx[c]) for c in range(N_CORES)])

    # self-loop diag indicators
    dinv2 = (dinv * dinv).astype(np.float32)
    node_of_row = np.empty(N_NODES_PAD, np.int64)
    node_of_row[zfrow] = np.arange(N_NODES_PAD)
    dinv2_row = dinv2[node_of_row]
    ind_self = np.zeros((N_CORES, P, BPC * P), np.float16)
    ar = np.arange(P)
    for c in range(N_CORES):
        for i in range(BPC):
            r0 = _zf_base(c, i * P)
            ind_self[c, ar, i * P + ar] = \
                dinv2_row[r0:r0 + P].astype(np.float16)

    layout = dict(cntA=cntA.astype(int), cntB=cntB.astype(int),
                  posA=posA, posB=posB, grp_cntA=grp_cntA.astype(int),
                  grp_cntB=grp_cntB.astype(int), C=C, n_groups=n_groups,
                  gstart=gstart, zfrow=zfrow)
    data = dict(eidx=eidx, ind_edge=ind_edge, ind_self=ind_self)
    return layout, data


def prepare_labels(edge_label_index, n_label, zfrow):
    """Bucket labels by (a-seg, b-seg) per core; (A,A) bucket first."""
    a = zfrow[np.asarray(edge_label_index[0], dtype=np.int64)]
    b = zfrow[np.asarray(edge_label_index[1], dtype=np.int64)]
    per = n_label // N_CORES
    buckets = []
    for c in range(N_CORES):
        la = a[c * per:(c + 1) * per]
        lb = b[c * per:(c + 1) * per]
        lab = np.arange(c * per, (c + 1) * per)
        bid = (la >= NA) * 2 + (lb >= NA)
        buckets.append([(la[bid == k], lb[bid == k], lab[bid == k])
                        for k in range(4)])
    tcnt = [max(int(np.ceil(len(buckets[c][k][0]) / P))
                for c in range(N_CORES)) for k in range(4)]
    T = sum(tcnt)
    aidx = np.zeros((N_CORES, T * P), np.int64)
    bidx = np.zeros((N_CORES, T * P), np.int64)
    labmap = np.full((N_CORES, T * P), -1, np.int64)
    for c in range(N_CORES):
        pos = 0
        for k in range(4):
            la, lb, lab = buckets[c][k]
            n = len(la)
            aidx[c, pos:pos + n] = la - (NA if k >= 2 else 0)
            bidx[c, pos:pos + n] = lb - (NA if k % 2 else 0)
            labmap[c, pos:pos + n] = lab
            pos += tcnt[k] * P
    la_s = np.stack([_wrap16(aidx[c]) for c in range(N_CORES)])
    lb_s = np.stack([_wrap16(bidx[c]) for c in range(N_CORES)])
    return dict(la=la_s, lb=lb_s, tcnt=tcnt, T=T, labmap=labmap)


# ------------------------------------------------------------- device kernel

def build_bass(lay, tcnt, in_c, hid_c, out_c, bias_zero):
    from concourse import bacc, bass, mybir
    import concourse.tile as tile

    C = lay["C"]
    T = int(sum(tcnt))
    n_groups = lay["n_groups"]
    gstart = lay["gstart"]
    cntA, cntB = lay["cntA"], lay["cntB"]
    posA, posB = lay["posA"], lay["posB"]
    grp_cntA, grp_cntB = lay["grp_cntA"], lay["grp_cntB"]
    f32 = mybir.dt.float32
    f16 = mybir.dt.float16
    i16 = mybir.dt.int16

    nc = bacc.Bacc("TRN2", target_bir_lowering=False, debug=False,
                   num_devices=N_CORES, num_swdge_queues=4)

    xf_d = nc.dram_tensor("xf", [N_NODES_PAD, in_c], f16,
                          kind="ExternalInput")
    xs_d = nc.dram_tensor("xs", [BPC * P, in_c], f16, kind="ExternalInput")
    w_d = [nc.dram_tensor(f"W{i+1}", s, f16, kind="ExternalInput")
           for i, s in enumerate([[in_c, hid_c], [hid_c, hid_c],
                                  [hid_c, out_c]])]
    b_d = [nc.dram_tensor(f"b{i+1}", [s], f16, kind="ExternalInput")
           for i, s in enumerate([hid_c, hid_c, out_c])]
    eidx_d = nc.dram_tensor("eidx", [P, C * 8], i16, kind="ExternalInput")
    inde_d = nc.dram_tensor("inde", [P, C * P], f16, kind="ExternalInput")
    inds_d = nc.dram_tensor("inds", [P, BPC * P], f16, kind="ExternalInput")
    la_d = nc.dram_tensor("la", [P, T * 8], i16, kind="ExternalInput")
    lb_d = nc.dram_tensor("lb", [P, T * 8], i16, kind="ExternalInput")
    out_d = nc.dram_tensor("out", [P, T], f32, kind="ExternalOutput")

    zs_d = [nc.dram_tensor(f"zs{l}", [BPC * P, w], dt, kind="Internal")
            for l, (w, dt) in enumerate([(hid_c, f16), (hid_c, f16),
                                         (out_c, f32)])]
    zfA_d = [nc.dram_tensor(f"zfA{l}", [NA, w], dt, kind="Internal",
                            addr_space="Shared")
             for l, (w, dt) in enumerate([(hid_c, f16), (hid_c, f16),
                                          (out_c, f32)])]
    zfB_d = [nc.dram_tensor(f"zfB{l}", [NB, w], dt, kind="Internal",
                            addr_space="Shared")
             for l, (w, dt) in enumerate([(hid_c, f16), (hid_c, f16),
                                          (out_c, f32)])]

    gq = [0]

    def next_q():
        q = gq[0]
        gq[0] = (q + 1) % 4
        return q

    rg = [list(range(N_CORES))]

    with tile.TileContext(nc) as tc:
        with (
            tc.tile_pool(name="consts", bufs=1) as cst,
            tc.tile_pool(name="gathA", bufs=5) as gpa,
            tc.tile_pool(name="gathB", bufs=3) as gpb,
            tc.tile_pool(name="indp", bufs=10) as ip,
            tc.tile_pool(name="selfg", bufs=4) as sp,
            tc.tile_pool(name="outp", bufs=4) as op,
            tc.tile_pool(name="dec", bufs=2) as dp,
            tc.tile_pool(name="psA", bufs=6, space="PSUM") as psA,
            tc.tile_pool(name="psZ", bufs=2, space="PSUM") as psZ,
        ):
            # ---- resident constants
            ones1 = cst.tile([1, P], f16)
            nc.vector.memset(ones1[:], 1.0)

            eidx_sb = cst.tile([P, C * 8], i16)
            nc.sync.dma_start(eidx_sb[:], eidx_d[:, :])
            inds_sb = cst.tile([P, BPC * P], f16)
            nc.sync.dma_start(inds_sb[:], inds_d[:, :])
            la_sb = cst.tile([P, T * 8], i16)
            nc.sync.dma_start(la_sb[:], la_d[:, :])
            lb_sb = cst.tile([P, T * 8], i16)
            nc.sync.dma_start(lb_sb[:], lb_d[:, :])

            w_sb, bias_sb = [], []
            for l in range(3):
                wt = cst.tile([hid_c if l else in_c,
                               out_c if l == 2 else hid_c], f16)
                nc.sync.dma_start(wt[:], w_d[l][:, :])
                w_sb.append(wt)
                bt = cst.tile([1, out_c if l == 2 else hid_c], f16)
                nc.sync.dma_start(bt[:], b_d[l][None, :])
                bias_sb.append(bt)

            def gather_calls(g3s, tab, c0, c1, nsplit):
                n = c1 - c0
                if n <= 0:
                    return
                step = (n + nsplit - 1) // nsplit
                for a in range(c0, c1, step):
                    b = min(a + step, c1)
                    nc.gpsimd.dma_gather(
                        out_ap=g3s[:, a - c0:b - c0, :],
                        in_ap=tab,
                        idxs_ap=eidx_sb[:, a * 8:b * 8],
                        num_idxs=(b - a) * P, num_idxs_reg=(b - a) * P,
                        elem_size=in_c, single_packet=False,
                        queue_num=next_q())

            # ---- 3 GCN layers
            for l in range(3):
                oc = out_c if l == 2 else hid_c
                ztype = f32 if l == 2 else f16
                A_tab = xf_d[:NA, :] if l == 0 else zfA_d[l - 1][:, :]
                B_tab = xf_d[NA:, :] if l == 0 else zfB_d[l - 1][:, :]
                prev = xs_d if l == 0 else zs_d[l - 1]

                g3a_q, g3b_q, it_q = {}, {}, {}

                def issue_A(g):
                    if g >= n_groups:
                        return
                    b0 = int(gstart[g])
                    gcA = int(grp_cntA[g])
                    baseA = int(posA[b0])
                    gta = gpa.tile([P, max(gcA, 1) * in_c], f16, tag="ga",
                                   name=f"ga{l}_{g}")
                    g3a = gta[:].rearrange("p (c f) -> p c f",
                                           c=max(gcA, 1))
                    gather_calls(g3a, A_tab, baseA, baseA + gcA, 2)
                    g3a_q[g] = g3a

                def issue_B(g):
                    if g >= n_groups:
                        return
                    b0 = int(gstart[g])
                    gcB = int(grp_cntB[g])
                    baseB = int(posB[b0])
                    gtb = gpb.tile([P, max(gcB, 1) * in_c], f16, tag="gb",
                                   name=f"gb{l}_{g}")
                    g3b = gtb[:].rearrange("p (c f) -> p c f",
                                           c=max(gcB, 1))
                    gather_calls(g3b, B_tab, baseB, baseB + gcB, 2)
                    g3b_q[g] = g3b

                def issue_ind(i):
                    if i >= BPC:
                        return
                    cA, cB = int(cntA[i]), int(cntB[i])
                    it = ip.tile([P, (cA + cB) * P], f16, tag="ind",
                                 name=f"it{l}_{i}")
                    pa, pb = int(posA[i]), int(posB[i])
                    nc.scalar.dma_start(
                        it[:, :cA * P], inde_d[:, pa * P:(pa + cA) * P])
                    nc.scalar.dma_start(
                        it[:, cA * P:], inde_d[:, pb * P:(pb + cB) * P])
                    it_q[i] = it

                issue_A(0)
                issue_A(1)
                issue_A(2)
                issue_B(0)
                issue_B(1)
                for i0 in range(8):
                    issue_ind(i0)

                for g in range(n_groups):
                    b0, b1 = int(gstart[g]), int(gstart[g + 1])
                    baseA = int(posA[b0])
                    baseB = int(posB[b0])
                    issue_A(g + 3)
                    issue_B(g + 2)
                    g3a, g3b = g3a_q.pop(g), g3b_q.pop(g)

                    for i in range(b0, b1):
                        issue_ind(i + 8)
                        it = it_q.pop(i)
                        cA, cB = int(cntA[i]), int(cntB[i])
                        selfg = sp.tile([P, in_c], f16, tag="selfg")
                        nc.sync.dma_start(selfg[:],
                                          prev[i * P:(i + 1) * P, :])
                        agg_ps = psA.tile([P, P], f32, tag="agg",
                                          space="PSUM")
                        nc.tensor.matmul(
                            out=agg_ps[:], lhsT=selfg[:],
                            rhs=inds_sb[:, i * P:(i + 1) * P],
                            start=True, stop=(cA + cB == 0))
                        for which, cnt, pos0, g3, cbase, ib in (
                                (0, cA, int(posA[i]), g3a, baseA, 0),
                                (1, cB, int(posB[i]), g3b, baseB, cA)):
                            for k in range(cnt):
                                ck = pos0 + k
                                last = (which == 1 or cB == 0) and \
                                       (k == cnt - 1)
                                nc.tensor.matmul(
                                    out=agg_ps[:],
                                    lhsT=g3[:, ck - cbase, :],
                                    rhs=it[:, (ib + k) * P:
                                           (ib + k + 1) * P],
                                    start=False, stop=last)

                        aggT = op.tile([P, P], f16, tag="aggT")
                        nc.vector.tensor_copy(out=aggT[:], in_=agg_ps[:])

                        z_ps = psZ.tile([P, oc], f32, tag="z",
                                        space="PSUM")
                        if not bias_zero[l]:
                            nc.tensor.matmul(out=z_ps[:], lhsT=ones1[:],
                                             rhs=bias_sb[l][:],
                                             start=True, stop=False)
                        nc.tensor.matmul(out=z_ps[:], lhsT=aggT[:],
                                         rhs=w_sb[l][:],
                                         start=bias_zero[l], stop=True)

                        z_sb = op.tile([P, oc], ztype, tag="z_sb")
                        if l < 2:
                            nc.vector.tensor_scalar_max(
                                out=z_sb[:], in0=z_ps[:], scalar1=0.0)
                        else:
                            nc.vector.tensor_copy(out=z_sb[:],
                                                  in_=z_ps[:])
                        nc.sync.dma_start(zs_d[l][i * P:(i + 1) * P, :],
                                          z_sb[:])

                    if b1 == SEG0_BLOCKS:
                        nc.gpsimd.collective_compute(
                            "AllGather", mybir.AluOpType.bypass,
                            replica_groups=rg,
                            ins=[zs_d[l][:SA, :]], outs=[zfA_d[l][:, :]])
                nc.gpsimd.collective_compute(
                    "AllGather", mybir.AluOpType.bypass,
                    replica_groups=rg,
                    ins=[zs_d[l][SA:, :]], outs=[zfB_d[l][:, :]])

            # ---- decode; bucket 0 = (A,A) depends only on zfA
            res = cst.tile([P, T], f32)
            tbase = 0
            for k in range(4):
                tk = int(tcnt[k])
                if tk == 0:
                    continue
                a_tab = zfB_d[2][:, :] if k >= 2 else zfA_d[2][:, :]
                b_tab = zfB_d[2][:, :] if k % 2 else zfA_d[2][:, :]
                for h0 in range(0, tk, 16):
                    hk = min(16, tk - h0)
                    ga = dp.tile([P, 16 * out_c], f32, tag="dga")
                    gb = dp.tile([P, 16 * out_c], f32, tag="dgb")
                    ga3 = ga[:, :hk * out_c].rearrange(
                        "p (c f) -> p c f", c=hk)
                    gb3 = gb[:, :hk * out_c].rearrange(
                        "p (c f) -> p c f", c=hk)
                    t0 = tbase + h0
                    nc.gpsimd.dma_gather(
                        out_ap=ga3, in_ap=a_tab,
                        idxs_ap=la_sb[:, t0 * 8:(t0 + hk) * 8],
                        num_idxs=hk * P, num_idxs_reg=hk * P,
                        elem_size=out_c, single_packet=False,
                        queue_num=next_q())
                    nc.gpsimd.dma_gather(
                        out_ap=gb3, in_ap=b_tab,
                        idxs_ap=lb_sb[:, t0 * 8:(t0 + hk) * 8],
                        num_idxs=hk * P, num_idxs_reg=hk * P,
                        elem_size=out_c, single_packet=False,
                        queue_num=next_q())
                    nc.vector.tensor_mul(out=ga[:, :hk * out_c],
                                         in0=ga[:, :hk * out_c],
                                         in1=gb[:, :hk * out_c])
                    nc.vector.tensor_reduce(
                        out=res[:, t0:t0 + hk], in_=ga3,
                        axis=mybir.AxisListType.X, op=mybir.AluOpType.add)
                tbase += tk
            nc.sync.dma_start(out_d[:, :], res[:])

    nc.finalize()
    return nc


# ---------------------------------------------------------------- entry point

def kernel(x, W1, b1, W2, b2, W3, b3, edge_index, edge_label_index):
    from concourse.bass_utils import run_bass_kernel_spmd

    x = np.asarray(x, dtype=np.float32)
    n_nodes, in_c = x.shape
    hid_c = np.asarray(W2).shape[0]
    out_c = np.asarray(W3).shape[1]
    n_label = np.asarray(edge_label_index).shape[1]

    lay, data = prepare(edge_index, n_nodes)
    lb = prepare_labels(edge_label_index, n_label, lay["zfrow"])

    bias_zero = [bool(np.all(np.asarray(b) == 0)) for b in (b1, b2, b3)]
    nc = build_bass(lay, lb["tcnt"], in_c, hid_c, out_c, bias_zero)

    xf = np.zeros((N_NODES_PAD, in_c), np.float16)
    xf[lay["zfrow"][:n_nodes]] = x[:n_nodes].astype(np.float16)

    common = {
        "xf": xf,
        "W1": np.asarray(W1).astype(np.float16),
        "W2": np.asarray(W2).astype(np.float16),
        "W3": np.asarray(W3).astype(np.float16),
        "b1": np.asarray(b1).astype(np.float16),
        "b2": np.asarray(b2).astype(np.float16),
        "b3": np.asarray(b3).astype(np.float16),
    }
    in_maps = []
    for c in range(N_CORES):
        m = dict(common)
        m["xs"] = np.ascontiguousarray(np.concatenate([
            xf[c * SA:(c + 1) * SA],
            xf[NA + c * SB:NA + (c + 1) * SB]]))
        m["eidx"] = np.ascontiguousarray(data["eidx"][c])
        m["inde"] = np.ascontiguousarray(data["ind_edge"][c])
        m["inds"] = np.ascontiguousarray(data["ind_self"][c])
        m["la"] = np.ascontiguousarray(lb["la"][c])
        m["lb"] = np.ascontiguousarray(lb["lb"][c])
        in_maps.append(m)

    res = run_bass_kernel_spmd(nc, in_maps, core_ids=list(range(N_CORES)))

    out = np.zeros((n_label,), np.float32)
    for c in range(N_CORES):
        o = res.results[c]["out"]  # [P, T]
        flat = o.T.reshape(-1)
        lm = lb["labmap"][c]
        valid = lm >= 0
        out[lm[valid]] = flat[valid]
    return out
